# revision 1
# baseline (speedup 1.0000x reference)
"""Trainium2 Bass kernel for nn_DirectionVarEntropy.

Computes, per 14x14 patch and channel:
  - pixel-value entropy (256-bin histogram of round(x*255))
  - direction variance psi of 3x3-DCT sliding-window directional stds
  - richness = mean_c(psi_m * entropy)  ->  output (B, Hp, Wp)

Sharding: pure data parallel over batch, 2 images per core on 8 cores.

Per-core layout: 2048 spatial patches x 3 channels = 6144 patch-channels,
mapped to [128 partitions x 48 free segments]; seg s = t*3 + c where
t = spatial_patch // 128, partition p = spatial_patch % 128.

Entropy (the histogram_binning part): instead of materializing 256-bin
histograms (which needs either scatter-add hardware this chip lacks, or
256 compare+reduce passes dominated by per-instruction overhead), compute
per-pixel own-bin counts c_p = #\{q: pi_q == pi_p\} with 195 circular-shift
tensor_tensor(is_equal) + add passes in bf16 (DVE 2x mode), each one
instruction covering all 48 segments.  Then
  E = log2(196) - mean_p ln(c_p)/ln 2
which equals the dense-histogram entropy up to the reference's 1e-10
epsilon terms (~1e-6 relative).  Shifted reads stay 4B-aligned via two
doubled pixel buffers (one rotated by a pixel) so the DVE keeps its fast
mode for odd shifts.

DCT part: explicit 9 coefficient planes via separable 3-tap convolutions
(tensor_scalar + scalar_tensor_tensor on shifted access patterns), group
sums / stds / psi in fp32 on DVE; ACT does squares and sqrt via
exp(0.5*ln x) so every activation stays in one LUT function-set (no
1.3us table reloads).  SBUF is phase-scoped: conv/psi blocks run first
(X + work pool), then the entropy phase reuses that space.
"""

import functools

import numpy as np

import concourse.bacc as bacc
import concourse.bass as bass
import concourse.mybir as mybir
from concourse import bass_utils
from concourse.tile import TileContext

P = 128
PH = 14
NWIN = 12          # sliding 3x3 positions per axis
NPIX = PH * PH     # 196
BINS = 256
LN2 = 0.6931471805599453
F32 = mybir.dt.float32
BF16 = mybir.dt.bfloat16
ALU = mybir.AluOpType
ACTF = mybir.ActivationFunctionType

# problem shape (hardcoded per contract)
B_FULL, C, H, W = 16, 3, 448, 448
N_CORES = 8
B_CORE = B_FULL // N_CORES      # 2
HP = H // PH                    # 32
T_BLKS = B_CORE * HP * HP // P  # 16 t-blocks of 128 spatial patches
SEGS = T_BLKS * C               # 48


def _build(dct_flat: tuple, segs: int = SEGS, bins: int = BINS,
           nb: int = 3, act_bins: int = 0) -> bass.Bass:
    """Build the SPMD single-core program. dct_flat: 9 floats, row major."""
    D = np.asarray(dct_flat, np.float64).reshape(3, 3)
    nc = bacc.Bacc("TRN2", debug=False, enable_asserts=False)

    x_d = nc.dram_tensor("x", (B_CORE, C, H, W), F32, kind="ExternalInput")
    out_d = nc.dram_tensor("out", (B_CORE, HP, HP), F32, kind="ExternalOutput")
    # (b, c, hp, i, wp, j) view of DRAM input, reordered to (b c hp wp i j)
    xv = x_d.ap().rearrange("b c (hp i) (wp j) -> b c hp wp i j", i=PH, j=PH)
    ov = out_d.ap()

    n_blocks = (segs + nb - 1) // nb

    with TileContext(nc) as tc:
        with tc.tile_pool(name="persist", bufs=1) as pp:
            X = pp.tile([P, segs, PH, PH], F32)
            Xf = X.rearrange("p s i j -> p (s i j)")
            TMP = pp.tile([P, (segs // 8) * NPIX], F32)
            dummy = pp.tile([P, NPIX], BF16)
            pdum = pp.tile([P, NWIN * NWIN], F32)
            psi_acc = pp.tile([P, segs], F32)
            e_acc = pp.tile([P, segs], F32)
            rich = pp.tile([P, segs], F32)
            rich3 = rich.rearrange("p (t c) -> p t c", c=C)
            tsum = pp.tile([P, segs // C], F32)
            osb = pp.tile([P, segs // C], F32)

            # ---- input DMAs: per (t, c, p1) a [32, 14, 14] strided load ----
            for t in range(T_BLKS):
                b = t // (T_BLKS // B_CORE)
                hp0 = (t % (T_BLKS // B_CORE)) * 4
                for c in range(C):
                    s = t * C + c
                    for p1 in range(4):
                        nc.sync.dma_start(
                            X[p1 * 32:(p1 + 1) * 32, s],
                            xv[b, c, hp0 + p1],
                        )
            # Per-DMA same-engine absorber copies: each waits on exactly one
            # DMA queue semaphore; all downstream DVE reads of X then order
            # behind these in program order (no multi-sem waits, which
            # overflow the ISA sync-wait slots).
            for t in range(T_BLKS):
                for c in range(C):
                    s = t * C + c
                    for p1 in range(4):
                        sl = X[p1 * 32:(p1 + 1) * 32, s]
                        nc.vector.tensor_copy(sl, sl)

            d = [[float(D[r, c]) for c in range(3)] for r in range(3)]

            wp_ctx = tc.tile_pool(name="work", bufs=2)
            wp = wp_ctx.__enter__()
            for blk in range(n_blocks):
                s0 = blk * nb
                sn = min(nb, segs - s0)
                # conv tiles for this block
                V = [wp.tile([P, nb, NWIN, PH], F32, tag=f"V{r}", name=f"V{r}")
                     for r in range(3)]
                Y = [[wp.tile([P, nb, NWIN, NWIN], F32, tag=f"Y{r}{c}", name=f"Y{r}{c}")
                      for c in range(3)] for r in range(3)]
                xb = X[:, s0:s0 + sn]

                # vertical convs V_r(i,j) = sum_k D[r,k] x(i+k, j)
                for r in range(3):
                    vb = V[r][:, :sn]
                    nc.vector.tensor_scalar(
                        vb, xb[:, :, 0:NWIN, :], d[r][0], None, ALU.mult)
                    for k in (1, 2):
                        nc.vector.scalar_tensor_tensor(
                            vb, xb[:, :, k:k + NWIN, :], d[r][k], vb,
                            ALU.mult, ALU.add)
                # horizontal convs Y_rc(i,j) = sum_l D[c,l] V_r(i, j+l)
                for r in range(3):
                    vb = V[r][:, :sn]
                    for c in range(3):
                        yb = Y[r][c][:, :sn]
                        nc.vector.tensor_scalar(
                            yb, vb[:, :, :, 0:NWIN], d[c][0], None, ALU.mult)
                        for l in (1, 2):
                            nc.vector.scalar_tensor_tensor(
                                yb, vb[:, :, :, l:l + NWIN], d[c][l], yb,
                                ALU.mult, ALU.add)

                # group sums of Y (pre-square): rows, cols, diag, anti-diag
                GROUPS = (
                    [[(r, 0), (r, 1), (r, 2)] for r in range(3)]       # rows
                    + [[(0, c), (1, c), (2, c)] for c in range(3)]     # cols
                    + [[(0, 0), (1, 1), (2, 2)],                       # diag
                       [(0, 2), (1, 1), (2, 0)]]                       # anti
                )
                M = [wp.tile([P, nb, NWIN, NWIN], F32, tag=f"M{g}", name=f"M{g}")
                     for g in range(8)]
                SS = [wp.tile([P, nb, NWIN, NWIN], F32, tag=f"SS{g}", name=f"SS{g}")
                      for g in range(8)]
                for g, mem in enumerate(GROUPS):
                    mb = M[g][:, :sn]
                    (r0, c0), (r1, c1), (r2, c2) = mem
                    nc.vector.tensor_add(
                        mb, Y[r0][c0][:, :sn], Y[r1][c1][:, :sn])
                    nc.vector.tensor_add(mb, mb, Y[r2][c2][:, :sn])
                    # Msq = (M/3)^2 in place
                    nc.scalar.activation(mb, mb, ACTF.Square, scale=1.0 / 3)
                # squares of Y in place
                for r in range(3):
                    for c in range(3):
                        yb = Y[r][c][:, :sn]
                        nc.scalar.activation(yb, yb, ACTF.Square)
                for g, mem in enumerate(GROUPS):
                    sb = SS[g][:, :sn]
                    (r0, c0), (r1, c1), (r2, c2) = mem
                    nc.vector.tensor_add(
                        sb, Y[r0][c0][:, :sn], Y[r1][c1][:, :sn])
                    nc.vector.tensor_add(sb, sb, Y[r2][c2][:, :sn])
                    # std^2 = SS/3 - (M/3)^2, clamp, sqrt -> sigma in SS tile
                    # (sqrt via exp(0.5*ln x): keeps every ACT func in the
                    # natural_log_exp_and_others table set -- no table swaps)
                    nc.vector.scalar_tensor_tensor(
                        sb, sb, 1.0 / 3, M[g][:, :sn], ALU.mult, ALU.subtract)
                    nc.vector.tensor_scalar_max(sb, sb, 1e-38)
                    nc.scalar.activation(sb, sb, ACTF.Ln)
                    nc.scalar.activation(sb, sb, ACTF.Exp, scale=0.5)

                U1 = wp.tile([P, nb, NWIN, NWIN], F32, tag="U1", name="U1")
                U2 = wp.tile([P, nb, NWIN, NWIN], F32, tag="U2", name="U2")
                t1 = wp.tile([P, nb, NWIN, NWIN], F32, tag="t1", name="t1")
                t2 = wp.tile([P, nb, NWIN, NWIN], F32, tag="t2", name="t2")
                A = wp.tile([P, nb, NWIN, NWIN], F32, tag="A", name="A")
                sum2 = wp.tile([P, nb, NWIN, NWIN], F32, tag="sum2", name="sum2")
                aq = wp.tile([P, nb, NWIN, NWIN], F32, tag="aq", name="aq")
                s_t = wp.tile([P, nb, NWIN, NWIN], F32, tag="s_t", name="s_t")
                ssq = wp.tile([P, nb, NWIN, NWIN], F32, tag="ssq", name="ssq")
                rinv = wp.tile([P, nb, NWIN, NWIN], F32, tag="rinv", name="rinv")
                psi = wp.tile([P, nb, NWIN, NWIN], F32, tag="psi", name="psi")
                u1, u2 = U1[:, :sn], U2[:, :sn]
                tb1, tb2 = t1[:, :sn], t2[:, :sn]
                ab = A[:, :sn]
                s2b, aqb = sum2[:, :sn], aq[:, :sn]
                stb, ssqb, rb, psib = (s_t[:, :sn], ssq[:, :sn],
                                       rinv[:, :sn], psi[:, :sn])
                sig = [SS[g][:, :sn] for g in range(8)]

                nc.vector.tensor_add(u1, sig[0], sig[1])
                nc.vector.tensor_add(u1, u1, sig[2])
                nc.vector.tensor_add(u2, sig[3], sig[4])
                nc.vector.tensor_add(u2, u2, sig[5])
                # A = U1/3 + U2/3 + sig6 + sig7
                nc.vector.scalar_tensor_tensor(
                    tb1, u1, 1.0 / 3, sig[6], ALU.mult, ALU.add)
                nc.vector.scalar_tensor_tensor(
                    tb2, u2, 1.0 / 3, sig[7], ALU.mult, ALU.add)
                nc.vector.tensor_add(ab, tb1, tb2)
                # sum of squared directional stds
                nc.scalar.activation(u1, u1, ACTF.Square, scale=1.0 / 3)
                nc.scalar.activation(u2, u2, ACTF.Square, scale=1.0 / 3)
                nc.scalar.activation(sig[6], sig[6], ACTF.Square)
                nc.scalar.activation(sig[7], sig[7], ACTF.Square)
                nc.vector.tensor_add(tb1, u1, u2)
                nc.vector.tensor_add(tb2, sig[6], sig[7])
                nc.vector.tensor_add(s2b, tb1, tb2)
                # psi = (sum2 - A^2/4) / (3 * (A/4 + 1e-8)^2)
                nc.scalar.activation(aqb, ab, ACTF.Square, scale=0.5)
                nc.vector.tensor_sub(s2b, s2b, aqb)
                nc.vector.tensor_scalar(
                    stb, ab, 0.25, 1e-8, ALU.mult, ALU.add)
                nc.scalar.activation(ssqb, stb, ACTF.Square)
                nc.vector.reciprocal(rb, ssqb)
                nc.vector.scalar_tensor_tensor(
                    psib, s2b, 1.0 / 3, rb, ALU.mult, ALU.mult)
                # psi_m accumulate per seg
                for i in range(sn):
                    s = s0 + i
                    nc.vector.tensor_scalar(
                        pdum, psib[:, i].rearrange("p i j -> p (i j)"),
                        1.0, None, ALU.mult, ALU.add,
                        accum_out=psi_acc[:, s:s + 1])

            wp_ctx.__exit__(None, None, None)
            ep_ctx = tc.tile_pool(name="ent", bufs=1)
            ep = ep_ctx.__enter__()
            # ---- quantize: pi = round(x*255) via the 2^23 RNE trick ----
            # PI2: per seg the 196 pixel codes stored twice (j and j+196) so
            # circularly shifted reads stay within the seg row.  PI2o: the
            # same, rotated by one pixel, so odd shifts read at even (4B)
            # offsets and keep the DVE 2x mode.
            PI2 = ep.tile([P, segs, 2 * NPIX], BF16)
            PI2o = ep.tile([P, segs, 2 * NPIX], BF16)
            TWO23 = float(2 ** 23)
            qch = (segs // 8) * NPIX
            TMP3 = TMP.rearrange("p (s k) -> p s k", k=NPIX)
            spq = segs // 8
            for q in range(8):
                nc.vector.tensor_scalar(
                    TMP, Xf[:, q * qch:(q + 1) * qch], 255.0, TWO23,
                    ALU.mult, ALU.add)
                nc.vector.tensor_scalar(
                    PI2[:, q * spq:(q + 1) * spq, 0:NPIX], TMP3, TWO23,
                    None, ALU.subtract)
            nc.vector.tensor_copy(PI2[:, :, NPIX:2 * NPIX],
                                  PI2[:, :, 0:NPIX])
            nc.vector.tensor_copy(PI2o[:, :, 0:2 * NPIX - 1],
                                  PI2[:, :, 1:2 * NPIX])
            nc.vector.tensor_copy(PI2o[:, :, 2 * NPIX - 1:2 * NPIX],
                                  PI2[:, :, 1:2])

            # ---- entropy: per-pixel own-bin counts via 195 shifted
            # equality passes (all segs per instruction), then
            # E = log2(N) - mean_p ln(count_p) / ln 2 ----
            ACC = ep.tile([P, segs, NPIX], BF16)
            EQT = ep.tile([P, segs, NPIX], BF16)
            base = PI2[:, :, 0:NPIX]
            ACCf = ACC.rearrange("p s k -> p (s k)")
            EQTf = EQT.rearrange("p s k -> p (s k)")
            nc.vector.tensor_tensor(ACC, base, PI2o[:, :, 0:NPIX],
                                    ALU.is_equal)
            for s in range(2, NPIX):
                if s % 2 == 0:
                    shifted = PI2[:, :, s:s + NPIX]
                else:
                    shifted = PI2o[:, :, s - 1:s - 1 + NPIX]
                nc.vector.tensor_tensor(EQT, base, shifted, ALU.is_equal)
                nc.vector.tensor_tensor(ACC, ACC, EQT, ALU.add)
            # ln(count) with the +1 self-match folded into the ACT bias
            LNP = ep.tile([P, segs, NPIX], F32)
            LNPf = LNP.rearrange("p s k -> p (s k)")
            nc.scalar.activation(LNPf, ACCf, ACTF.Ln, bias=1.0)
            for s in range(segs):
                nc.vector.tensor_scalar(
                    dummy, LNP[:, s], 1.0, None, ALU.mult,
                    ALU.add, accum_out=e_acc[:, s:s + 1])

            ep_ctx.__exit__(None, None, None)
            # ---- richness = psi_m * entropy, mean over channels ----
            import math
            nc.vector.tensor_scalar(
                e_acc, e_acc, -1.0 / (NPIX * LN2), float(math.log2(NPIX)),
                ALU.mult, ALU.add)
            nc.vector.scalar_tensor_tensor(
                rich, psi_acc, 1.0 / (NWIN * NWIN), e_acc,
                ALU.mult, ALU.mult)
            nc.vector.tensor_add(tsum, rich3[:, :, 0], rich3[:, :, 1])
            nc.vector.tensor_add(tsum, tsum, rich3[:, :, 2])
            nc.vector.tensor_scalar(osb, tsum, 1.0 / C, None, ALU.mult)

            # ---- output DMAs ----
            for t in range(T_BLKS):
                b = t // (T_BLKS // B_CORE)
                hp0 = (t % (T_BLKS // B_CORE)) * 4
                nc.sync.dma_start(ov[b, hp0:hp0 + 4], osb[:, t:t + 1])

    nc.compile()
    return nc


@functools.lru_cache(maxsize=4)
def _build_cached(dct_flat: tuple) -> bass.Bass:
    return _build(dct_flat)


def kernel(x, dct_matrix):
    x = np.ascontiguousarray(np.asarray(x, dtype=np.float32))
    D = np.asarray(dct_matrix, dtype=np.float32)
    assert x.shape == (B_FULL, C, H, W), x.shape
    nc = _build_cached(tuple(float(v) for v in D.flatten()))
    in_maps = [
        {"x": np.ascontiguousarray(x[i * B_CORE:(i + 1) * B_CORE])}
        for i in range(N_CORES)
    ]
    res = bass_utils.run_bass_kernel_spmd(
        nc, in_maps, core_ids=list(range(N_CORES)))
    out = np.concatenate([r["out"] for r in res.results], axis=0)
    return out.astype(np.float32)



# revision 10
# speedup vs baseline: 1.2011x; 1.2011x over previous
"""Trainium2 Bass kernel for nn_DirectionVarEntropy.

Computes, per 14x14 patch and channel:
  - pixel-value entropy (256-bin histogram of round(x*255))
  - direction variance psi of 3x3-DCT sliding-window directional stds
  - richness = mean_c(psi_m * entropy)  ->  output (B, Hp, Wp)

Sharding: pure data parallel over batch, 2 images per core on 8 cores.

Per-core layout: 2048 spatial patches x 3 channels = 6144 patch-channels,
mapped to [128 partitions x 48 free segments]; seg s = t*3 + c where
t = spatial_patch // 128, partition p = spatial_patch % 128.

Entropy: instead of the O(N^2) shifted-equality scheme, sort each seg's
196 pixel codes (bitonic merge network, 36 min/max stages in bf16, pads
of 320 to width 256), then recover per-pixel own-bin counts c_p from run
lengths: boundary-index arrays + running max (run start) and reversed
running min (next run start) via custom DVE scan ops, c_p = nxt[p+1] -
start[p].  E = log2(196) - mean_p ln(c_p)/ln 2, identical to the dense
histogram entropy up to the reference's 1e-10 epsilon terms.

DCT part: vertical 3-tap convs write transposed (j-major) V planes so the
horizontal convs read contiguous rank-3 views and can use a fused 2-tap
custom DVE op; group sums run on the Pool engine in parallel; sigma^2 =
max(SS/3 - Msq, eps) is one fused custom op; ACT does squares and sqrt
via exp(0.5*ln x) so every activation stays in one LUT function-set.
"""

import functools
import math

import numpy as np

import concourse.bacc as bacc
import concourse.bass as bass
import concourse.mybir as mybir
from concourse import bass_utils
from concourse.tile import TileContext

# ---------------- custom DVE ops (registered at import) ----------------
import concourse.dve_ops as dve_ops
from concourse.dve_spec import (Spec, Src0, Src1, C0, C1, Zero, MaxNeg,
                                eq, maxx, select, scan, lower as dve_lower,
                                AluOp, Idx, _has_src1)
from concourse.dve_uop import DveOpSpec
from concourse.bass import BassVectorEngine


def _register(name: str, spec: Spec, subdim: bool = False):
    for op in dve_ops.OPS:
        if op.name == name:
            return op
    row = dve_ops._CUSTOM_DVE_ROW_BASE + len(dve_ops.OPS)
    assert row < 0x20, "custom DVE op rows exhausted"
    shas = {}
    for ver in ("v3", "v4"):
        s = DveOpSpec(name=name, opcode=row, uops=dve_lower(spec, ver=ver),
                      rd1_en=_has_src1(spec))
        shas[ver] = s.sha(ver)
    op = dve_ops.DveOp(name, spec, subdim=subdim, uops_sha=shas)
    dve_ops.OPS.append(op)
    dve_ops._SUB_OPCODE_FOR_NAME[name] = row
    dve_ops.CUSTOM_DVE_SPECS[name] = spec
    return op


def _np_bidx(fill):
    def ref(in0, in1, s0, s1, imm2):
        n = int(np.prod(in0.shape[1:]))
        idx = np.arange(n, dtype=np.float32).reshape((1,) + in0.shape[1:])
        return np.where(in0 == in1, np.float32(fill), idx).astype(np.float32)
    return ref


_BIDX_NEG = _register(
    "ATH_BIDX_NEG",
    Spec(body=select(eq(Src0, Src1), MaxNeg, Idx),
         reference=_np_bidx(-3.4028235e38)))
_BIDX_POS = _register(
    "ATH_BIDX_POS",
    Spec(body=select(eq(Src0, Src1), Zero - MaxNeg, Idx),
         reference=_np_bidx(3.4028235e38)))
_SCAN_MAX = _register(
    "ATH_SCAN_MAX",
    Spec(body=scan(AluOp.MAX, Src0),
         reference=lambda in0, in1, s0, s1, imm2:
         np.maximum.accumulate(
             in0.reshape(in0.shape[0], -1), axis=1).reshape(in0.shape)))
_SCAN_MIN = _register(
    "ATH_SCAN_MIN",
    Spec(body=scan(AluOp.MIN, Src0, init=C0),
         reference=lambda in0, in1, s0, s1, imm2:
         np.minimum.accumulate(
             np.minimum(in0, s0).reshape(in0.shape[0], -1),
             axis=1).reshape(in0.shape)))
_CONV2 = _register(
    "ATH_CONV2",
    Spec(body=Src0 * C0 + Src1 * C1,
         reference=lambda in0, in1, s0, s1, imm2: in0 * s0 + in1 * s1))
_VARM = _register(
    "ATH_VARM",
    Spec(body=maxx(Src0 * C0 - Src1, C1),
         reference=lambda in0, in1, s0, s1, imm2:
         np.maximum(in0 * s0 - in1, s1)))


def _v_bidx_neg(self, out, in0, in1):
    return self._custom_dve(_BIDX_NEG, out=out, in0=in0, in1=in1)


def _v_bidx_pos(self, out, in0, in1):
    return self._custom_dve(_BIDX_POS, out=out, in0=in0, in1=in1)


def _v_scan_max(self, out, in0):
    return self._custom_dve(_SCAN_MAX, out=out, in0=in0)


def _v_scan_min(self, out, in0, init):
    return self._custom_dve(_SCAN_MIN, out=out, in0=in0, s0=init)


def _v_conv2(self, out, in0, in1, c0, c1):
    return self._custom_dve(_CONV2, out=out, in0=in0, in1=in1, s0=c0, s1=c1)


def _v_varm(self, out, in0, in1, scale, clamp):
    return self._custom_dve(_VARM, out=out, in0=in0, in1=in1,
                            s0=scale, s1=clamp)


BassVectorEngine.ath_bidx_neg = _v_bidx_neg
BassVectorEngine.ath_bidx_pos = _v_bidx_pos
BassVectorEngine.ath_scan_max = _v_scan_max
BassVectorEngine.ath_scan_min = _v_scan_min
BassVectorEngine.ath_conv2 = _v_conv2
BassVectorEngine.ath_varm = _v_varm

# ---------------- problem constants ----------------
P = 128
PH = 14
NWIN = 12          # sliding 3x3 positions per axis
NPIX = PH * PH     # 196
LN2 = 0.6931471805599453
F32 = mybir.dt.float32
BF16 = mybir.dt.bfloat16
ALU = mybir.AluOpType
ACTF = mybir.ActivationFunctionType

B_FULL, C, H, W = 16, 3, 448, 448
N_CORES = 8
B_CORE = B_FULL // N_CORES      # 2
HP = H // PH                    # 32
T_BLKS = B_CORE * HP * HP // P  # 16 t-blocks of 128 spatial patches
SEGS = T_BLKS * C               # 48

SW = 256                        # sort width per seg (padded)
PADV = 320.0                    # pad value > max code 255
SDOM = NPIX + 1                 # scan domain per seg (196 codes + 1 pad)
BIG = 3.0e38


def _emit_sort(nc, SRT, Bp, segs=SEGS):
    """Bitonic merge-sort (ascending) of SRT[:, :, 1:1+SW] (bf16), ping-pong
    with Bp.  36 stages; even count -> result lands back in SRT."""
    ping = lambda: SRT[:, :, 1:1 + SW]
    pong = lambda: Bp[:, :, :]
    cur_in, cur_out = ping, pong
    nstage = 0
    nphase = SW.bit_length() - 1          # 8
    for j in range(nphase):
        m = 2 << j
        h = m // 2
        ain, aout = cur_in(), cur_out()
        i4 = ain.rearrange("p s (nb m) -> p s nb m", m=m)
        o4 = aout.rearrange("p s (nb m) -> p s nb m", m=m)
        lo_in = i4[:, :, :, 0:h]
        hi_in = i4[:, :, :, h:m]
        if h > 1:
            hi_rev = i4[:, :, :, m - 1:h - 1:-1]
            lo_rev = i4[:, :, :, h - 1::-1]
        else:
            hi_rev = i4[:, :, :, m - 1:m]
            lo_rev = i4[:, :, :, 0:1]
        nc.vector.tensor_tensor(o4[:, :, :, 0:h], lo_in, hi_rev, ALU.min)
        nc.vector.tensor_tensor(o4[:, :, :, h:m], hi_in, lo_rev, ALU.max)
        cur_in, cur_out = cur_out, cur_in
        nstage += 1
        d = h // 2
        while d >= 1:
            ain, aout = cur_in(), cur_out()
            i4 = ain.rearrange("p s (nb t) -> p s nb t", t=2 * d)
            o4 = aout.rearrange("p s (nb t) -> p s nb t", t=2 * d)
            nc.vector.tensor_tensor(
                o4[:, :, :, 0:d], i4[:, :, :, 0:d], i4[:, :, :, d:2 * d],
                ALU.min)
            nc.vector.tensor_tensor(
                o4[:, :, :, d:2 * d], i4[:, :, :, 0:d], i4[:, :, :, d:2 * d],
                ALU.max)
            cur_in, cur_out = cur_out, cur_in
            nstage += 1
            d //= 2
    assert nstage == 36 and cur_in == ping, (nstage,)


def _build(dct_flat: tuple, nb: int = 3) -> bass.Bass:
    """Build the SPMD single-core program. dct_flat: 9 floats, row major."""
    D = np.asarray(dct_flat, np.float64).reshape(3, 3)
    nc = bacc.Bacc("TRN2", debug=False, enable_asserts=False)

    x_d = nc.dram_tensor("x", (B_CORE, C, H, W), F32, kind="ExternalInput")
    out_d = nc.dram_tensor("out", (B_CORE, HP, HP), F32, kind="ExternalOutput")
    xv = x_d.ap().rearrange("b c (hp i) (wp j) -> b c hp wp i j", i=PH, j=PH)
    ov = out_d.ap()

    segs = SEGS
    n_blocks = (segs + nb - 1) // nb
    d = [[float(D[r, c]) for c in range(3)] for r in range(3)]

    with TileContext(nc) as tc:
        with tc.tile_pool(name="persist", bufs=1) as pp:
            X = pp.tile([P, segs, PH, PH], F32)
            Xf = X.rearrange("p s i j -> p (s i j)")
            TMP = pp.tile([P, (segs // 8) * NPIX], F32)
            pdum = pp.tile([P, NWIN * NWIN], F32)
            lnd = pp.tile([P, NPIX], F32)
            psi_acc = pp.tile([P, segs], F32)
            e_acc = pp.tile([P, segs], F32)
            rich = pp.tile([P, segs], F32)
            rich3 = rich.rearrange("p (t c) -> p t c", c=C)
            tsum = pp.tile([P, segs // C], F32)
            osb = pp.tile([P, segs // C], F32)

            # ---- input DMAs: per (t, c, p1) a [32, 14, 14] strided load ----
            for t in range(T_BLKS):
                b = t // (T_BLKS // B_CORE)
                hp0 = (t % (T_BLKS // B_CORE)) * 4
                for c in range(C):
                    s = t * C + c
                    for p1 in range(4):
                        nc.sync.dma_start(
                            X[p1 * 32:(p1 + 1) * 32, s],
                            xv[b, c, hp0 + p1],
                        )
            # Per-DMA same-engine absorber copies (see baseline notes).
            for t in range(T_BLKS):
                for c in range(C):
                    s = t * C + c
                    for p1 in range(4):
                        sl = X[p1 * 32:(p1 + 1) * 32, s]
                        nc.vector.tensor_copy(sl, sl)

            # ================= entropy phase =================
            ep_ctx = tc.tile_pool(name="ent", bufs=1)
            ep = ep_ctx.__enter__()
            SRT = ep.tile([P, segs, 1 + SW], BF16)
            Bp = ep.tile([P, segs, SW], BF16)
            S = ep.tile([P, segs * SDOM], F32)
            M = ep.tile([P, (segs // 2) * SDOM], F32)
            NXT = ep.tile([P, (segs // 2) * SDOM], F32)

            # quantize: codes = round(x*255) via the 2^23 RNE trick, into
            # SRT cols 1..197 (bf16, exact for ints <= 256)
            TWO23 = float(2 ** 23)
            qch = (segs // 8) * NPIX
            TMP3 = TMP.rearrange("p (s k) -> p s k", k=NPIX)
            spq = segs // 8
            for q in range(8):
                nc.vector.tensor_scalar(
                    TMP, Xf[:, q * qch:(q + 1) * qch], 255.0, TWO23,
                    ALU.mult, ALU.add)
                nc.vector.tensor_scalar(
                    SRT[:, q * spq:(q + 1) * spq, 1:1 + NPIX], TMP3, TWO23,
                    None, ALU.subtract)
            # sentinel col 0 = -1; pad cols 197..256 = PADV
            nc.vector.tensor_scalar(
                SRT[:, :, 0:1], X[:, :, 0:1, 0:1], 0.0, -1.0,
                ALU.mult, ALU.add)
            Xsk = Xf.rearrange("p (s k) -> p s k", k=NPIX)
            nc.vector.tensor_scalar(
                SRT[:, :, 1 + NPIX:1 + SW],
                Xsk[:, :, 0:SW - NPIX], 0.0, PADV,
                ALU.mult, ALU.add)

            _emit_sort(nc, SRT, Bp)

            # run-length counts in two halves of 24 segs
            hs = segs // 2
            Sv = S.rearrange("p (s k) -> p s k", k=SDOM)
            for hf in range(2):
                s0 = hf * hs
                src0 = SRT[:, s0:s0 + hs, 1:1 + SDOM]
                src1 = SRT[:, s0:s0 + hs, 0:SDOM]
                Sh = S[:, s0 * SDOM:(s0 + hs) * SDOM]
                nc.vector.ath_bidx_neg(M, src0, src1)
                nc.vector.ath_scan_max(Sh, M)
                nc.vector.ath_bidx_pos(M, src0, src1)
                n = hs * SDOM
                nc.vector.ath_scan_min(NXT[:, n - 1::-1], M[:, n - 1::-1],
                                       BIG)
                # c[f] = NXT[f+1] - S[f], in place into S (codes rows only)
                NXTv = NXT.rearrange("p (s k) -> p s k", k=SDOM)
                nc.vector.tensor_tensor(
                    Sv[:, s0:s0 + hs, 0:NPIX],
                    NXTv[:, :, 1:1 + NPIX],
                    Sv[:, s0:s0 + hs, 0:NPIX], ALU.subtract)
            # ln + per-seg accumulate on ACT
            for s in range(segs):
                nc.scalar.activation(
                    lnd, Sv[:, s, 0:NPIX], ACTF.Ln,
                    accum_out=e_acc[:, s:s + 1])

            ep_ctx.__exit__(None, None, None)

            # ================= conv / psi phase =================
            wp_ctx = tc.tile_pool(name="work", bufs=2)
            wp = wp_ctx.__enter__()
            for blk in range(n_blocks):
                s0 = blk * nb
                sn = min(nb, segs - s0)
                V = [wp.tile([P, nb, NWIN, PH], F32, tag=f"V{r}", name=f"V{r}")
                     for r in range(3)]
                Y = [[wp.tile([P, nb, NWIN * NWIN], F32, tag=f"Y{r}{c}",
                              name=f"Y{r}{c}")
                      for c in range(3)] for r in range(3)]
                xb = X[:, s0:s0 + sn]

                # vertical convs V_r(i,j) = sum_k D[r,k] x(i+k, j)
                for r in range(3):
                    vb = V[r][:, :sn]
                    nc.vector.tensor_scalar(
                        vb, xb[:, :, 0:NWIN, :], d[r][0], None, ALU.mult)
                    for k in (1, 2):
                        nc.vector.scalar_tensor_tensor(
                            vb, xb[:, :, k:k + NWIN, :], d[r][k], vb,
                            ALU.mult, ALU.add)
                # horizontal convs Y_rc(i,j) = sum_l D[c,l] V_r(i, j+l)
                for r in range(3):
                    vb = V[r][:, :sn]
                    for c in range(3):
                        yb = Y[r][c][:, :sn].rearrange(
                            "p n (i j) -> p n i j", j=NWIN)
                        nc.vector.tensor_scalar(
                            yb, vb[:, :, :, 0:NWIN], d[c][0], None, ALU.mult)
                        for l in (1, 2):
                            nc.vector.scalar_tensor_tensor(
                                yb, vb[:, :, :, l:l + NWIN], d[c][l], yb,
                                ALU.mult, ALU.add)

                GROUPS = (
                    [[(r, 0), (r, 1), (r, 2)] for r in range(3)]       # rows
                    + [[(0, c), (1, c), (2, c)] for c in range(3)]     # cols
                    + [[(0, 0), (1, 1), (2, 2)],                       # diag
                       [(0, 2), (1, 1), (2, 0)]]                       # anti
                )
                M8 = [wp.tile([P, nb, NWIN * NWIN], F32, tag=f"M{g}",
                              name=f"M{g}") for g in range(8)]
                SS = [wp.tile([P, nb, NWIN * NWIN], F32, tag=f"SS{g}",
                              name=f"SS{g}") for g in range(8)]
                # group sums of Y (pre-square) on Pool
                for g, mem in enumerate(GROUPS):
                    mb = M8[g][:, :sn]
                    (r0, c0), (r1, c1), (r2, c2) = mem
                    nc.gpsimd.tensor_add(
                        mb, Y[r0][c0][:, :sn], Y[r1][c1][:, :sn])
                    nc.gpsimd.tensor_add(mb, mb, Y[r2][c2][:, :sn])
                    # Msq = (M/3)^2 in place (ACT)
                    nc.scalar.activation(mb, mb, ACTF.Square, scale=1.0 / 3)
                # squares of Y in place (ACT)
                for r in range(3):
                    for c in range(3):
                        yb = Y[r][c][:, :sn]
                        nc.scalar.activation(yb, yb, ACTF.Square)
                for g, mem in enumerate(GROUPS):
                    sb = SS[g][:, :sn]
                    (r0, c0), (r1, c1), (r2, c2) = mem
                    nc.vector.tensor_add(
                        sb, Y[r0][c0][:, :sn], Y[r1][c1][:, :sn])
                    nc.vector.tensor_add(sb, sb, Y[r2][c2][:, :sn])
                    # sigma^2 = max(SS/3 - Msq, eps), then sigma via ACT
                    # exp(0.5*ln x) (stays in one LUT set)
                    nc.vector.ath_varm(sb, sb, M8[g][:, :sn], 1.0 / 3, 1e-38)
                    nc.scalar.activation(sb, sb, ACTF.Ln)
                    nc.scalar.activation(sb, sb, ACTF.Exp, scale=0.5)

                U1 = wp.tile([P, nb, NWIN * NWIN], F32, tag="U1", name="U1")
                U2 = wp.tile([P, nb, NWIN * NWIN], F32, tag="U2", name="U2")
                t1 = wp.tile([P, nb, NWIN * NWIN], F32, tag="t1", name="t1")
                t2 = wp.tile([P, nb, NWIN * NWIN], F32, tag="t2", name="t2")
                A = wp.tile([P, nb, NWIN * NWIN], F32, tag="A", name="A")
                sum2 = wp.tile([P, nb, NWIN * NWIN], F32, tag="sum2",
                               name="sum2")
                aq = wp.tile([P, nb, NWIN * NWIN], F32, tag="aq", name="aq")
                s_t = wp.tile([P, nb, NWIN * NWIN], F32, tag="s_t", name="s_t")
                ssq = wp.tile([P, nb, NWIN * NWIN], F32, tag="ssq", name="ssq")
                rinv = wp.tile([P, nb, NWIN * NWIN], F32, tag="rinv",
                               name="rinv")
                psi = wp.tile([P, nb, NWIN * NWIN], F32, tag="psi", name="psi")
                u1, u2 = U1[:, :sn], U2[:, :sn]
                tb1, tb2 = t1[:, :sn], t2[:, :sn]
                ab = A[:, :sn]
                s2b, aqb = sum2[:, :sn], aq[:, :sn]
                stb, ssqb, rb, psib = (s_t[:, :sn], ssq[:, :sn],
                                       rinv[:, :sn], psi[:, :sn])
                sig = [SS[g][:, :sn] for g in range(8)]

                nc.gpsimd.tensor_add(u1, sig[0], sig[1])
                nc.gpsimd.tensor_add(u1, u1, sig[2])
                nc.gpsimd.tensor_add(u2, sig[3], sig[4])
                nc.gpsimd.tensor_add(u2, u2, sig[5])
                # A = U1/3 + U2/3 + sig6 + sig7
                nc.vector.scalar_tensor_tensor(
                    tb1, u1, 1.0 / 3, sig[6], ALU.mult, ALU.add)
                nc.vector.scalar_tensor_tensor(
                    tb2, u2, 1.0 / 3, sig[7], ALU.mult, ALU.add)
                nc.vector.tensor_add(ab, tb1, tb2)
                # sum of squared directional stds
                nc.scalar.activation(u1, u1, ACTF.Square, scale=1.0 / 3)
                nc.scalar.activation(u2, u2, ACTF.Square, scale=1.0 / 3)
                nc.scalar.activation(sig[6], sig[6], ACTF.Square)
                nc.scalar.activation(sig[7], sig[7], ACTF.Square)
                nc.gpsimd.tensor_add(tb1, u1, u2)
                nc.gpsimd.tensor_add(tb2, sig[6], sig[7])
                nc.vector.tensor_add(s2b, tb1, tb2)
                # psi = (sum2 - A^2/4) / (3 * (A/4 + 1e-8)^2)
                nc.scalar.activation(aqb, ab, ACTF.Square, scale=0.5)
                nc.vector.tensor_sub(s2b, s2b, aqb)
                nc.vector.tensor_scalar(
                    stb, ab, 0.25, 1e-8, ALU.mult, ALU.add)
                nc.scalar.activation(ssqb, stb, ACTF.Square)
                nc.vector.reciprocal(rb, ssqb)
                nc.vector.scalar_tensor_tensor(
                    psib, s2b, 1.0 / 3, rb, ALU.mult, ALU.mult)
                # psi_m accumulate per seg
                for i in range(sn):
                    s = s0 + i
                    nc.vector.tensor_scalar(
                        pdum, psib[:, i], 1.0, None, ALU.mult,
                        ALU.add, accum_out=psi_acc[:, s:s + 1])

            wp_ctx.__exit__(None, None, None)

            # ---- richness = psi_m * entropy, mean over channels ----
            nc.vector.tensor_scalar(
                e_acc, e_acc, -1.0 / (NPIX * LN2), float(math.log2(NPIX)),
                ALU.mult, ALU.add)
            nc.vector.scalar_tensor_tensor(
                rich, psi_acc, 1.0 / (NWIN * NWIN), e_acc,
                ALU.mult, ALU.mult)
            nc.vector.tensor_add(tsum, rich3[:, :, 0], rich3[:, :, 1])
            nc.vector.tensor_add(tsum, tsum, rich3[:, :, 2])
            nc.vector.tensor_scalar(osb, tsum, 1.0 / C, None, ALU.mult)

            # ---- output DMAs ----
            for t in range(T_BLKS):
                b = t // (T_BLKS // B_CORE)
                hp0 = (t % (T_BLKS // B_CORE)) * 4
                nc.sync.dma_start(ov[b, hp0:hp0 + 4], osb[:, t:t + 1])

    nc.compile()
    return nc


@functools.lru_cache(maxsize=4)
def _build_cached(dct_flat: tuple) -> bass.Bass:
    return _build(dct_flat)


def kernel(x, dct_matrix):
    x = np.ascontiguousarray(np.asarray(x, dtype=np.float32))
    D = np.asarray(dct_matrix, dtype=np.float32)
    assert x.shape == (B_FULL, C, H, W), x.shape
    nc = _build_cached(tuple(float(v) for v in D.flatten()))
    in_maps = [
        {"x": np.ascontiguousarray(x[i * B_CORE:(i + 1) * B_CORE])}
        for i in range(N_CORES)
    ]
    res = bass_utils.run_bass_kernel_spmd(
        nc, in_maps, core_ids=list(range(N_CORES)))
    out = np.concatenate([r["out"] for r in res.results], axis=0)
    return out.astype(np.float32)


# revision 75
# speedup vs baseline: 4.7329x; 3.9404x over previous
"""Trainium2 Bass kernel for nn_DirectionVarEntropy.

Computes, per 14x14 patch and channel:
  - pixel-value entropy (256-bin histogram of round(x*255))
  - direction variance psi of 3x3-DCT sliding-window directional stds
  - richness = mean_c(psi_m * entropy)  ->  output (B, Hp, Wp)

Sharding: pure data parallel over batch, 2 images per core on 8 cores.

Per-core layout: 2048 spatial patches x 3 channels = 6144 patch-channels,
mapped to [128 partitions x 48 free segments]; seg s = t*3 + c where
t = spatial_patch // 128, partition p = spatial_patch % 128.

Entropy: sort each seg's 196 pixel codes (bitonic merge network, 36
min/max stages in bf16 at the DVE 2x rate, pads of 320 to width 256),
then per-pixel own-bin counts c_p from run lengths: boundary-index
select + running max (run start) and reversed running min (next run
start) via custom DVE scan ops, c_p = nxt[p+1] - start[p].
E = log2(196) - mean_p ln(c_p)/ln 2 == the dense-histogram entropy up
to the reference's 1e-10 epsilon terms.  The 36 sort stages are
interleaved into the conv/psi block loop as DVE filler so cross-engine
dependency bubbles are spent on sorting instead of idling.

DCT part: 3-tap separable convs with a fused 2-tap custom DVE op on
rank-3 coalesced views; Y planes / squares / SS sums in bf16 (2x DVE
rate; sigma^2 = SS/3 - Msq is safe because SS and M derive from the
same rounded Y, so rounding largely cancels in the variance); group
sums on the Pool engine; sigma^2 fused in one custom op, sigma via ACT
Sqrt (same LUT set as Square, so the only table switch is the final
Ln).  psi is emitted one block late (software pipelining) so the next
block's convs are not queued behind cross-engine psi stalls; the
run-length scans run as 8 double-buffered 6-seg chunks after the loop.
"""

import functools
import math

import numpy as np

import concourse.bacc as bacc
import concourse.bass as bass
import concourse.mybir as mybir
from concourse import bass_utils
from concourse.tile import TileContext

# ---------------- custom DVE ops (registered at import) ----------------
import concourse.dve_ops as dve_ops
from concourse.dve_spec import (Spec, Src0, Src1, C0, C1, Zero, MaxNeg,
                                eq, maxx, select, scan, lower as dve_lower,
                                AluOp, Idx, _has_src1)
from concourse.dve_uop import DveOpSpec
from concourse.bass import BassVectorEngine


def _register(name: str, spec: Spec, subdim: bool = False):
    for op in dve_ops.OPS:
        if op.name == name:
            return op
    row = dve_ops._CUSTOM_DVE_ROW_BASE + len(dve_ops.OPS)
    assert row < 0x20, "custom DVE op rows exhausted"
    shas = {}
    for ver in ("v3", "v4"):
        s = DveOpSpec(name=name, opcode=row, uops=dve_lower(spec, ver=ver),
                      rd1_en=_has_src1(spec))
        shas[ver] = s.sha(ver)
    op = dve_ops.DveOp(name, spec, subdim=subdim, uops_sha=shas)
    dve_ops.OPS.append(op)
    dve_ops._SUB_OPCODE_FOR_NAME[name] = row
    dve_ops.CUSTOM_DVE_SPECS[name] = spec
    return op


def _np_bidx(fill):
    def ref(in0, in1, s0, s1, imm2):
        n = int(np.prod(in0.shape[1:]))
        idx = np.arange(n, dtype=np.float32).reshape((1,) + in0.shape[1:])
        return np.where(in0 == in1, np.float32(fill), idx).astype(np.float32)
    return ref


_BIDX_NEG = _register(
    "ATH_BIDX_NEG",
    Spec(body=select(eq(Src0, Src1), MaxNeg, Idx),
         reference=_np_bidx(-3.4028235e38)))
_BIDX_POS = _register(
    "ATH_BIDX_POS",
    Spec(body=select(eq(Src0, Src1), Zero - MaxNeg, Idx),
         reference=_np_bidx(3.4028235e38)))
_SCAN_MAX = _register(
    "ATH_SCAN_MAX",
    Spec(body=scan(AluOp.MAX, Src0),
         reference=lambda in0, in1, s0, s1, imm2:
         np.maximum.accumulate(
             in0.reshape(in0.shape[0], -1), axis=1).reshape(in0.shape)))
_SCAN_MIN = _register(
    "ATH_SCAN_MIN",
    Spec(body=scan(AluOp.MIN, Src0, init=C0),
         reference=lambda in0, in1, s0, s1, imm2:
         np.minimum.accumulate(
             np.minimum(in0, s0).reshape(in0.shape[0], -1),
             axis=1).reshape(in0.shape)))
_CONV2 = _register(
    "ATH_CONV2",
    Spec(body=Src0 * C0 + Src1 * C1,
         reference=lambda in0, in1, s0, s1, imm2: in0 * s0 + in1 * s1))
_VARM = _register(
    "ATH_VARM",
    Spec(body=maxx(Src0 * C0 - Src1, C1),
         reference=lambda in0, in1, s0, s1, imm2:
         np.maximum(in0 * s0 - in1, s1)))


def _v_bidx_neg(self, out, in0, in1):
    return self._custom_dve(_BIDX_NEG, out=out, in0=in0, in1=in1)


def _v_bidx_pos(self, out, in0, in1):
    return self._custom_dve(_BIDX_POS, out=out, in0=in0, in1=in1)


def _v_scan_max(self, out, in0):
    return self._custom_dve(_SCAN_MAX, out=out, in0=in0)


def _v_scan_min(self, out, in0, init):
    return self._custom_dve(_SCAN_MIN, out=out, in0=in0, s0=init)


def _v_conv2(self, out, in0, in1, c0, c1):
    return self._custom_dve(_CONV2, out=out, in0=in0, in1=in1, s0=c0, s1=c1)


def _v_varm(self, out, in0, in1, scale, clamp):
    return self._custom_dve(_VARM, out=out, in0=in0, in1=in1,
                            s0=scale, s1=clamp)


BassVectorEngine.ath_bidx_neg = _v_bidx_neg
BassVectorEngine.ath_bidx_pos = _v_bidx_pos
BassVectorEngine.ath_scan_max = _v_scan_max
BassVectorEngine.ath_scan_min = _v_scan_min
BassVectorEngine.ath_conv2 = _v_conv2
BassVectorEngine.ath_varm = _v_varm

# ---------------- problem constants ----------------
P = 128
PH = 14
NWIN = 12          # sliding 3x3 positions per axis
NPIX = PH * PH     # 196
LN2 = 0.6931471805599453
F32 = mybir.dt.float32
BF16 = mybir.dt.bfloat16
ALU = mybir.AluOpType
ACTF = mybir.ActivationFunctionType

B_FULL, C, H, W = 16, 3, 448, 448
N_CORES = 8
B_CORE = B_FULL // N_CORES      # 2
HP = H // PH                    # 32
T_BLKS = B_CORE * HP * HP // P  # 16 t-blocks of 128 spatial patches
SEGS = T_BLKS * C               # 48

SW = 256                        # sort width per seg (padded)
PADV = 320.0                    # pad value > max code 255
SDOM = NPIX + 1                 # scan domain per seg (196 codes + 1 pad)
BIG = 3.0e38


def _sort_stages(nc, SRT, Bp):
    """Generator yielding one bitonic stage (2 DVE instrs) per next().
    (Pool cannot run min/max TensorTensor -- ISA opcode check.)

    Each phase only processes the first ceil(196/m) merge blocks: the
    trailing blocks are all-PADV (initial layout puts pads at the end
    and in-block sorting keeps them there), so they are trivially
    sorted.  Requires both ping and pong buffers to have their pad
    columns pre-set to PADV: the skipped region is never written, so
    later phases read the preserved pad values."""
    ping = lambda: SRT[:, :, 1:1 + SW]
    pong = lambda: Bp[:, :, :]
    cur_in, cur_out = ping, pong

    def cmpex(lo_out, hi_out, lo_a, lo_b, hi_a, hi_b):
        nc.vector.tensor_tensor(lo_out, lo_a, lo_b, ALU.min)
        nc.vector.tensor_tensor(hi_out, hi_a, hi_b, ALU.max)

    nphase = SW.bit_length() - 1          # 8
    for j in range(nphase):
        m = 2 << j
        h = m // 2
        na = (NPIX + m - 1) // m          # active merge blocks
        nfull = NPIX // m                 # blocks fully below the boundary
        ain, aout = cur_in(), cur_out()
        i4 = ain.rearrange("p s (nb m) -> p s nb m", m=m)[:, :, 0:na]
        o4 = aout.rearrange("p s (nb m) -> p s nb m", m=m)[:, :, 0:na]

        def tri(blo, bhi, kmin, kmax):
            """Triangle compare-exchange for blocks [blo:bhi], min lanes
            [0:kmin], max lanes [0:kmax] (positions >= 196 are PADV at
            every stage and need no writes)."""
            ii = i4[:, :, blo:bhi]
            oo = o4[:, :, blo:bhi]
            if h > 1:
                hr = ii[:, :, :, m - 1:h - 1:-1]
                lr = ii[:, :, :, h - 1::-1]
            else:
                hr = ii[:, :, :, m - 1:m]
                lr = ii[:, :, :, 0:1]
            if kmin > 0:
                nc.vector.tensor_tensor(
                    oo[:, :, :, 0:kmin], ii[:, :, :, 0:kmin],
                    hr[:, :, :, 0:kmin], ALU.min)
            if kmax > 0:
                nc.vector.tensor_tensor(
                    oo[:, :, :, h:h + kmax], ii[:, :, :, h:h + kmax],
                    lr[:, :, :, 0:kmax], ALU.max)

        if nfull > 0:
            tri(0, nfull, h, h)
        if na > nfull:                    # boundary block, partial lanes
            base = nfull * m
            tri(nfull, na, min(h, NPIX - base),
                max(0, NPIX - base - h))
        cur_in, cur_out = cur_out, cur_in
        yield
        d = h // 2
        while d >= 1:
            # positions >= 196 hold PADV at EVERY stage (pads are global
            # maxima: each comparator's max lands at or above its inputs),
            # so sub-blocks based at >= 196 are droppable outright
            nc_act = (NPIX + 2 * d - 1) // (2 * d)
            ain, aout = cur_in(), cur_out()
            i4 = ain.rearrange("p s (nb t) -> p s nb t",
                               t=2 * d)[:, :, 0:nc_act]
            o4 = aout.rearrange("p s (nb t) -> p s nb t",
                                t=2 * d)[:, :, 0:nc_act]
            cmpex(o4[:, :, :, 0:d], o4[:, :, :, d:2 * d],
                  i4[:, :, :, 0:d], i4[:, :, :, d:2 * d],
                  i4[:, :, :, 0:d], i4[:, :, :, d:2 * d])
            cur_in, cur_out = cur_out, cur_in
            yield
            d //= 2
    assert cur_in == ping


def _build(dct_flat: tuple, nb: int = 3) -> bass.Bass:
    """Build the SPMD single-core program. dct_flat: 9 floats, row major."""
    D = np.asarray(dct_flat, np.float64).reshape(3, 3)
    nc = bacc.Bacc("TRN2", debug=False, enable_asserts=False)

    x_d = nc.dram_tensor("x", (B_CORE, C, H, W), F32, kind="ExternalInput")
    out_d = nc.dram_tensor("out", (B_CORE, HP, HP), F32, kind="ExternalOutput")
    xv = x_d.ap().rearrange("b c (hp i) (wp j) -> b c hp wp i j", i=PH, j=PH)
    ov = out_d.ap()

    segs = SEGS
    n_blocks = (segs + nb - 1) // nb
    d = [[float(D[r, c]) for c in range(3)] for r in range(3)]

    with TileContext(nc) as tc:
        with tc.tile_pool(name="persist", bufs=1) as pp:
            X = pp.tile([P, segs, PH, PH], F32)
            Xf = X.rearrange("p s i j -> p (s i j)")
            TMP = pp.tile([P, (segs // 8) * NPIX], F32)
            pdum = pp.tile([P, NWIN * NWIN], F32)
            lnd = pp.tile([P, NPIX], F32)
            psi_acc = pp.tile([P, segs], F32)
            e_acc = pp.tile([P, segs], F32)
            rich = pp.tile([P, segs], F32)
            rich3 = rich.rearrange("p (t c) -> p t c", c=C)
            tsum = pp.tile([P, segs // C], F32)
            osb = pp.tile([P, segs // C], F32)

            # ---- input DMAs: per (t, c, p1) a [32, 14, 14] strided load ----
            for t in range(T_BLKS):
                b = t // (T_BLKS // B_CORE)
                hp0 = (t % (T_BLKS // B_CORE)) * 4
                for c in range(C):
                    s = t * C + c
                    for p1 in range(4):
                        nc.sync.dma_start(
                            X[p1 * 32:(p1 + 1) * 32, s],
                            xv[b, c, hp0 + p1],
                        )
            # Per-seg absorber copies (each waits the 4 DMA queue sems of its
            # seg -- within the engine wait-queue depth); downstream reads of
            # X order behind these: DVE by program order, ACT via one
            # engine-counter semaphore.
            for t in range(T_BLKS):
                for c in range(C):
                    s = t * C + c
                    nc.vector.tensor_copy(X[:, s], X[:, s])

            ep_ctx = tc.tile_pool(name="ent", bufs=1)
            ep = ep_ctx.__enter__()
            SRT = ep.tile([P, segs, 1 + SW], BF16)
            b23p = pp.tile([P, 1], F32)
            b23n = pp.tile([P, 1], F32)
            nc.vector.memset(b23p, float(2 ** 23))
            nc.vector.memset(b23n, -float(2 ** 23))

            # quantize on ACT: codes = round(x*255) via the 2^23 RNE trick
            # (two Identity activations; Identity is in the same LUT set as
            # Ln/Exp/Square so no table reloads), into SRT cols 1..197
            # (bf16, exact for ints <= 256)
            TWO23 = float(2 ** 23)
            qch = (segs // 8) * NPIX
            TMP3 = TMP.rearrange("p (s k) -> p s k", k=NPIX)
            spq = segs // 8
            for q in range(8):
                # pass 1 (the RNE-rounding add) must be exact fp32 -> DVE;
                # pass 2 subtracts off 2^23 from integer-valued fp32 (exact
                # in any precision) -> ACT
                nc.vector.tensor_scalar(
                    TMP, Xf[:, q * qch:(q + 1) * qch], 255.0, TWO23,
                    ALU.mult, ALU.add)
                nc.scalar.activation(
                    SRT[:, q * spq:(q + 1) * spq, 1:1 + NPIX], TMP3,
                    ACTF.Identity, bias=b23n)
            # sentinel col 0 = -1; pad cols 197..256 = PADV
            nc.vector.memset(SRT[:, :, 0:1], -1.0)
            nc.vector.memset(SRT[:, :, 1 + NPIX:1 + SW], PADV)

            # ============ conv/psi blocks with interleaved sort ============
            wp_ctx = tc.tile_pool(name="work", bufs=2)
            wp = wp_ctx.__enter__()
            Bp = wp.tile([P, segs, SW], BF16, tag="BP", name="BP", bufs=1)
            # pads in the pong buffer too: the trailing all-pad merge
            # blocks are skipped by every sort stage, so both buffers
            # must carry PADV there from the start
            nc.gpsimd.memset(Bp[:, :, NPIX:SW], PADV)
            sorter = _sort_stages(nc, SRT, Bp)
            sort_left = 36

            def emit_sort(k):
                nonlocal sort_left
                for _ in range(min(k, sort_left)):
                    next(sorter)
                    sort_left -= 1

            GROUPS = (
                [[(r, 0), (r, 1), (r, 2)] for r in range(3)]       # rows
                + [[(0, c), (1, c), (2, c)] for c in range(3)]     # cols
                + [[(0, 0), (1, 1), (2, 2)],                       # diag
                   [(0, 2), (1, 1), (2, 0)]]                       # anti
            )

            def emit_psi(st):
                """psi for a previous block -- emitted one block late so the
                next block's convs aren't queued behind the psi stalls.
                fp32 temps alias the M8 tiles (last read by the VARMs) plus
                a dedicated 5-slot fp32 scratch; sig (bf16) is read-only so
                the sum2 - A^2/4 cancellation stays consistent in fp32."""
                s0, sn, M8, SS, PT = st
                u1, u2 = M8[0], M8[1]
                tb1, tb2 = M8[2], M8[3]
                ab, s2b = M8[4], M8[5]
                aqb, stb = M8[6], M8[7]
                s6q, s7q = PT[:, :, 0], PT[:, :, 1]
                # stb -> ssqb -> rb -> psib all reuse one slot in place
                stb = ssqb = rb = psib = PT[:, :, 2]
                sig = SS

                nc.gpsimd.tensor_add(u1, sig[0], sig[1])
                nc.gpsimd.tensor_add(u1, u1, sig[2])
                nc.gpsimd.tensor_add(u2, sig[3], sig[4])
                nc.gpsimd.tensor_add(u2, u2, sig[5])
                # A = U1/3 + U2/3 + sig6 + sig7
                nc.vector.scalar_tensor_tensor(
                    tb1, u1, 1.0 / 3, sig[6], ALU.mult, ALU.add)
                nc.vector.scalar_tensor_tensor(
                    tb2, u2, 1.0 / 3, sig[7], ALU.mult, ALU.add)
                nc.gpsimd.tensor_add(ab, tb1, tb2)
                # sum of squared directional stds
                nc.scalar.activation(u1, u1, ACTF.Square, scale=1.0 / 3)
                nc.scalar.activation(u2, u2, ACTF.Square, scale=1.0 / 3)
                nc.scalar.activation(s6q, sig[6], ACTF.Square)
                nc.scalar.activation(s7q, sig[7], ACTF.Square)
                nc.gpsimd.tensor_add(tb1, u1, u2)
                nc.gpsimd.tensor_add(tb2, s6q, s7q)
                nc.gpsimd.tensor_add(s2b, tb1, tb2)
                # psi = (sum2 - A^2/4) / (3 * (A/4 + 1e-8)^2)
                nc.scalar.activation(aqb, ab, ACTF.Square, scale=0.5)
                nc.gpsimd.tensor_sub(s2b, s2b, aqb)
                nc.vector.tensor_scalar(
                    stb, ab, 0.25, 1e-8, ALU.mult, ALU.add)
                nc.scalar.activation(ssqb, stb, ACTF.Square)
                nc.vector.reciprocal(rb, ssqb)
                nc.vector.scalar_tensor_tensor(
                    psib, s2b, 1.0 / 3, rb, ALU.mult, ALU.mult)
                # psi_m accumulate per seg on ACT (Identity + accum_out)
                for i in range(sn):
                    s = s0 + i
                    nc.scalar.activation(
                        pdum, psib[:, i], ACTF.Identity,
                        accum_out=psi_acc[:, s:s + 1])

            # run-length scan chunks (6 segs each, 8 chunks) emitted as DVE
            # filler into the last blocks once the sort has finished
            cs = segs // 8

            def emit_scan_chunk(h):
                s0c = h * cs
                src0 = SRT[:, s0c:s0c + cs, 1:1 + SDOM]
                src1 = SRT[:, s0c:s0c + cs, 0:SDOM]
                SC = wp.tile([P, cs * SDOM], F32, tag="SCS", name="SCS",
                             bufs=2)
                MC = wp.tile([P, cs * SDOM], F32, tag="SCM", name="SCM",
                             bufs=1)
                NX = wp.tile([P, cs * SDOM], F32, tag="SCN", name="SCN",
                             bufs=1)
                nc.vector.ath_bidx_neg(MC, src0, src1)
                nc.vector.ath_scan_max(SC, MC)
                nc.vector.ath_bidx_pos(MC, src0, src1)
                n = cs * SDOM
                nc.vector.ath_scan_min(NX[:, n - 1::-1], MC[:, n - 1::-1],
                                       BIG)
                Svc = SC.rearrange("p (s k) -> p s k", k=SDOM)
                NXv = NX.rearrange("p (s k) -> p s k", k=SDOM)
                # c[f] = NXT[f+1] - S[f], in place (codes rows only)
                nc.gpsimd.tensor_tensor(
                    Svc[:, :, 0:NPIX], NXv[:, :, 1:1 + NPIX],
                    Svc[:, :, 0:NPIX], ALU.subtract)
                for i in range(cs):
                    nc.scalar.activation(
                        lnd, Svc[:, i, 0:NPIX], ACTF.Ln,
                        accum_out=e_acc[:, s0c + i:s0c + i + 1])

            prev = None
            for blk in range(n_blocks):
                s0 = blk * nb
                sn = min(nb, segs - s0)
                assert sn == nb, "segs must divide by nb"
                V = [wp.tile([P, nb, NWIN, PH], BF16, tag=f"V{r}",
                             name=f"V{r}", bufs=1) for r in range(3)]
                Y = [[wp.tile([P, nb, NWIN * NWIN], BF16, tag=f"Y{r}{c}",
                              name=f"Y{r}{c}")
                      for c in range(3)] for r in range(3)]
                xbf = X[:, s0:s0 + sn].rearrange("p n i j -> p n (i j)")

                # convs: V and Y in bf16 (horizontal third taps then run at
                # the DVE 2x rate; SS and M derive from the same rounded Y
                # so the rounding cancels in sigma^2)
                with nc.allow_low_precision(
                        reason="bf16 V/Y conv planes; SS and M derive from "
                               "the same rounded Y so rounding cancels in "
                               "sigma^2"):
                    # vertical convs V_r(i,j) = sum_k D[r,k] x(i+k, j) on
                    # rank-3 coalesced views of the flat (i j) layout
                    for r in range(3):
                        vf = V[r].rearrange("p n i j -> p n (i j)")
                        nc.vector.ath_conv2(
                            vf, xbf[:, :, 0:NWIN * PH],
                            xbf[:, :, PH:PH * 13], d[r][0], d[r][1])
                        nc.vector.scalar_tensor_tensor(
                            vf, xbf[:, :, 2 * PH:PH * 14], d[r][2], vf,
                            ALU.mult, ALU.add)
                    # horizontal convs Y_rc(i,j) = sum_l D[c,l] V_r(i, j+l)
                    # on (n i)-coalesced rank-3 views
                    for r in range(3):
                        v3 = V[r].rearrange("p n i j -> p (n i) j")
                        for c in range(3):
                            y3 = Y[r][c].rearrange(
                                "p n (i j) -> p (n i) j", j=NWIN)
                            nc.vector.ath_conv2(
                                y3, v3[:, :, 0:NWIN], v3[:, :, 1:1 + NWIN],
                                d[c][0], d[c][1])
                            nc.vector.scalar_tensor_tensor(
                                y3, v3[:, :, 2:2 + NWIN], d[c][2], y3,
                                ALU.mult, ALU.add)

                M8T = wp.tile([P, nb, 8, NWIN * NWIN], F32, tag="M8T",
                              name="M8T")
                SST = wp.tile([P, nb, 8, NWIN * NWIN], BF16, tag="SST",
                              name="SST", bufs=1)
                M8 = [M8T[:, :, g] for g in range(8)]
                SS = [SST[:, :, g] for g in range(8)]
                # squares of Y (ACT) -- queued ahead of Msq so the SS sums
                # unblock early; bf16 so the SS sums run at the DVE 2x rate
                # (sigma^2 = SS/3 - Msq stays safe: SS and M derive from the
                # same rounded Y, so rounding mostly cancels in the variance)
                sq = wp.tile([P, nb, 9, NWIN * NWIN], BF16, tag="SQ",
                             name="sq", bufs=1)
                for r in range(3):
                    for c in range(3):
                        nc.scalar.activation(sq[:, :, r * 3 + c], Y[r][c],
                                             ACTF.Square)
                # psi of the previous block fills the conv->Msq gap
                if prev is not None:
                    emit_psi(prev)
                # group sums of Y: first pair-add on DVE in bf16 (2x, cheap),
                # final add on Pool -- halves the Pool chain ahead of
                # Msq -> VARM -> Sqrt; Msq = (M/3)^2 (ACT)
                mt = wp.tile([P, nb, 8, NWIN * NWIN], BF16, tag="MT",
                             name="mt", bufs=1)
                with nc.allow_low_precision(
                        reason="bf16 partial group sums; one extra rounding "
                               "on M, averaged out over 144 windows"):
                    for g, mem in enumerate(GROUPS):
                        nc.vector.tensor_add(
                            mt[:, :, g], Y[mem[0][0]][mem[0][1]],
                            Y[mem[1][0]][mem[1][1]])
                for g, mem in enumerate(GROUPS):
                    mb = M8[g]
                    nc.gpsimd.tensor_add(mb, mt[:, :, g],
                                         Y[mem[2][0]][mem[2][1]])
                    nc.scalar.activation(mb, mb, ACTF.Square, scale=1.0 / 3)
                emit_sort(1)
                with nc.allow_low_precision(
                        reason="SS sums in bf16: sigma^2 = SS/3 - Msq uses "
                               "the same rounded Y on both sides, rounding "
                               "cancels in the variance; errors average out "
                               "over 144 windows"):
                    for g, mem in enumerate(GROUPS):
                        sb = SS[g]
                        nc.vector.tensor_add(
                            sb, sq[:, :, mem[0][0] * 3 + mem[0][1]],
                            sq[:, :, mem[1][0] * 3 + mem[1][1]])
                        nc.vector.tensor_add(
                            sb, sb, sq[:, :, mem[2][0] * 3 + mem[2][1]])
                    # sigma^2 = max(SS/3 - Msq, eps) and sigma = sqrt, each
                    # as ONE instruction over the contiguous 8-group tiles
                    # (same LUT set as Square -- one table switch, at the
                    # final Ln)
                    SSf = SST.rearrange("p n g k -> p n (g k)")
                    M8f = M8T.rearrange("p n g k -> p n (g k)")
                    nc.vector.ath_varm(SSf, SSf, M8f, 1.0 / 3, 1e-38)
                    nc.scalar.activation(SSf, SSf, ACTF.Sqrt)
                emit_sort(1)
                PT = wp.tile([P, nb, 3, NWIN * NWIN], F32, tag="PT",
                             name="PT")
                prev = (s0, sn, M8, SS, PT)

            emit_psi(prev)
            emit_sort(36)
            for h in range(8):
                emit_scan_chunk(h)
            wp_ctx.__exit__(None, None, None)
            ep_ctx.__exit__(None, None, None)

            # ---- richness = psi_m * entropy, mean over channels ----
            nc.vector.tensor_scalar(
                e_acc, e_acc, -1.0 / (NPIX * LN2), float(math.log2(NPIX)),
                ALU.mult, ALU.add)
            nc.vector.scalar_tensor_tensor(
                rich, psi_acc, 1.0 / (NWIN * NWIN), e_acc,
                ALU.mult, ALU.mult)
            nc.vector.tensor_add(tsum, rich3[:, :, 0], rich3[:, :, 1])
            nc.vector.tensor_add(tsum, tsum, rich3[:, :, 2])
            nc.vector.tensor_scalar(osb, tsum, 1.0 / C, None, ALU.mult)

            # ---- output DMAs ----
            for t in range(T_BLKS):
                b = t // (T_BLKS // B_CORE)
                hp0 = (t % (T_BLKS // B_CORE)) * 4
                nc.sync.dma_start(ov[b, hp0:hp0 + 4], osb[:, t:t + 1])

    nc.compile()
    return nc


@functools.lru_cache(maxsize=4)
def _build_cached(dct_flat: tuple) -> bass.Bass:
    return _build(dct_flat)


def kernel(x, dct_matrix):
    x = np.ascontiguousarray(np.asarray(x, dtype=np.float32))
    D = np.asarray(dct_matrix, dtype=np.float32)
    assert x.shape == (B_FULL, C, H, W), x.shape
    nc = _build_cached(tuple(float(v) for v in D.flatten()))
    in_maps = [
        {"x": np.ascontiguousarray(x[i * B_CORE:(i + 1) * B_CORE])}
        for i in range(N_CORES)
    ]
    res = bass_utils.run_bass_kernel_spmd(
        nc, in_maps, core_ids=list(range(N_CORES)))
    out = np.concatenate([r["out"] for r in res.results], axis=0)
    return out.astype(np.float32)


# revision 76
# speedup vs baseline: 5.1669x; 1.0917x over previous
"""Trainium2 Bass kernel for nn_DirectionVarEntropy.

Computes, per 14x14 patch and channel:
  - pixel-value entropy (256-bin histogram of round(x*255))
  - direction variance psi of 3x3-DCT sliding-window directional stds
  - richness = mean_c(psi_m * entropy)  ->  output (B, Hp, Wp)

Sharding: pure data parallel over batch, 2 images per core on 8 cores.

Per-core layout: 2048 spatial patches x 3 channels = 6144 patch-channels,
mapped to [128 partitions x 48 free segments]; seg s = t*3 + c where
t = spatial_patch // 128, partition p = spatial_patch % 128.

Entropy: sort each seg's 196 pixel codes (bitonic merge network, 36
min/max stages in bf16 at the DVE 2x rate, pads of 320 to width 256),
then per-pixel own-bin counts c_p from run lengths: boundary-index
select + running max (run start) and reversed running min (next run
start) via custom DVE scan ops, c_p = nxt[p+1] - start[p].
E = log2(196) - mean_p ln(c_p)/ln 2 == the dense-histogram entropy up
to the reference's 1e-10 epsilon terms.  The 36 sort stages are
interleaved into the conv/psi block loop as DVE filler so cross-engine
dependency bubbles are spent on sorting instead of idling.

DCT part: 3-tap separable convs with a fused 2-tap custom DVE op on
rank-3 coalesced views; Y planes / squares / SS sums in bf16 (2x DVE
rate; sigma^2 = SS/3 - Msq is safe because SS and M derive from the
same rounded Y, so rounding largely cancels in the variance); group
sums on the Pool engine; sigma^2 fused in one custom op, sigma via ACT
Sqrt (same LUT set as Square, so the only table switch is the final
Ln).  psi is emitted one block late (software pipelining) so the next
block's convs are not queued behind cross-engine psi stalls; the
run-length scans run as 8 double-buffered 6-seg chunks after the loop.
"""

import functools
import math

import numpy as np

import concourse.bacc as bacc
import concourse.bass as bass
import concourse.mybir as mybir
from concourse import bass_utils
from concourse.tile import TileContext

# ---------------- custom DVE ops (registered at import) ----------------
import concourse.dve_ops as dve_ops
from concourse.dve_spec import (Spec, Src0, Src1, C0, C1, Zero, MaxNeg,
                                eq, maxx, select, scan, lower as dve_lower,
                                AluOp, Idx, _has_src1)
from concourse.dve_uop import DveOpSpec
from concourse.bass import BassVectorEngine


def _register(name: str, spec: Spec, subdim: bool = False):
    for op in dve_ops.OPS:
        if op.name == name:
            return op
    row = dve_ops._CUSTOM_DVE_ROW_BASE + len(dve_ops.OPS)
    assert row < 0x20, "custom DVE op rows exhausted"
    shas = {}
    for ver in ("v3", "v4"):
        s = DveOpSpec(name=name, opcode=row, uops=dve_lower(spec, ver=ver),
                      rd1_en=_has_src1(spec))
        shas[ver] = s.sha(ver)
    op = dve_ops.DveOp(name, spec, subdim=subdim, uops_sha=shas)
    dve_ops.OPS.append(op)
    dve_ops._SUB_OPCODE_FOR_NAME[name] = row
    dve_ops.CUSTOM_DVE_SPECS[name] = spec
    return op


def _np_bidx(fill):
    def ref(in0, in1, s0, s1, imm2):
        n = int(np.prod(in0.shape[1:]))
        idx = np.arange(n, dtype=np.float32).reshape((1,) + in0.shape[1:])
        return np.where(in0 == in1, np.float32(fill), idx).astype(np.float32)
    return ref


_BIDX_NEG = _register(
    "ATH_BIDX_NEG",
    Spec(body=select(eq(Src0, Src1), MaxNeg, Idx),
         reference=_np_bidx(-3.4028235e38)))
_BIDX_POS = _register(
    "ATH_BIDX_POS",
    Spec(body=select(eq(Src0, Src1), Zero - MaxNeg, Idx),
         reference=_np_bidx(3.4028235e38)))
_SCAN_MAX = _register(
    "ATH_SCAN_MAX",
    Spec(body=scan(AluOp.MAX, Src0),
         reference=lambda in0, in1, s0, s1, imm2:
         np.maximum.accumulate(
             in0.reshape(in0.shape[0], -1), axis=1).reshape(in0.shape)))
_SCAN_MIN = _register(
    "ATH_SCAN_MIN",
    Spec(body=scan(AluOp.MIN, Src0, init=C0),
         reference=lambda in0, in1, s0, s1, imm2:
         np.minimum.accumulate(
             np.minimum(in0, s0).reshape(in0.shape[0], -1),
             axis=1).reshape(in0.shape)))
_CONV2 = _register(
    "ATH_CONV2",
    Spec(body=Src0 * C0 + Src1 * C1,
         reference=lambda in0, in1, s0, s1, imm2: in0 * s0 + in1 * s1))
_VARM = _register(
    "ATH_VARM",
    Spec(body=maxx(Src0 * C0 - Src1, C1),
         reference=lambda in0, in1, s0, s1, imm2:
         np.maximum(in0 * s0 - in1, s1)))


def _v_bidx_neg(self, out, in0, in1):
    return self._custom_dve(_BIDX_NEG, out=out, in0=in0, in1=in1)


def _v_bidx_pos(self, out, in0, in1):
    return self._custom_dve(_BIDX_POS, out=out, in0=in0, in1=in1)


def _v_scan_max(self, out, in0):
    return self._custom_dve(_SCAN_MAX, out=out, in0=in0)


def _v_scan_min(self, out, in0, init):
    return self._custom_dve(_SCAN_MIN, out=out, in0=in0, s0=init)


def _v_conv2(self, out, in0, in1, c0, c1):
    return self._custom_dve(_CONV2, out=out, in0=in0, in1=in1, s0=c0, s1=c1)


def _v_varm(self, out, in0, in1, scale, clamp):
    return self._custom_dve(_VARM, out=out, in0=in0, in1=in1,
                            s0=scale, s1=clamp)


BassVectorEngine.ath_bidx_neg = _v_bidx_neg
BassVectorEngine.ath_bidx_pos = _v_bidx_pos
BassVectorEngine.ath_scan_max = _v_scan_max
BassVectorEngine.ath_scan_min = _v_scan_min
BassVectorEngine.ath_conv2 = _v_conv2
BassVectorEngine.ath_varm = _v_varm

# ---------------- problem constants ----------------
P = 128
PH = 14
NWIN = 12          # sliding 3x3 positions per axis
NPIX = PH * PH     # 196
LN2 = 0.6931471805599453
F32 = mybir.dt.float32
BF16 = mybir.dt.bfloat16
ALU = mybir.AluOpType
ACTF = mybir.ActivationFunctionType

B_FULL, C, H, W = 16, 3, 448, 448
N_CORES = 8
B_CORE = B_FULL // N_CORES      # 2
HP = H // PH                    # 32
T_BLKS = B_CORE * HP * HP // P  # 16 t-blocks of 128 spatial patches
SEGS = T_BLKS * C               # 48

SW = 256                        # sort width per seg (padded)
PADV = 320.0                    # pad value > max code 255
SDOM = NPIX + 1                 # scan domain per seg (196 codes + 1 pad)
BIG = 3.0e38


def _sort_stages(nc, SRT, Bp):
    """Generator yielding one bitonic stage (2 DVE instrs) per next().
    (Pool cannot run min/max TensorTensor -- ISA opcode check.)

    Each phase only processes the first ceil(196/m) merge blocks: the
    trailing blocks are all-PADV (initial layout puts pads at the end
    and in-block sorting keeps them there), so they are trivially
    sorted.  Requires both ping and pong buffers to have their pad
    columns pre-set to PADV: the skipped region is never written, so
    later phases read the preserved pad values."""
    ping = lambda: SRT[:, :, 1:1 + SW]
    pong = lambda: Bp[:, :, :]
    cur_in, cur_out = ping, pong

    def cmpex(lo_out, hi_out, lo_a, lo_b, hi_a, hi_b):
        nc.vector.tensor_tensor(lo_out, lo_a, lo_b, ALU.min)
        nc.vector.tensor_tensor(hi_out, hi_a, hi_b, ALU.max)

    nphase = SW.bit_length() - 1          # 8
    for j in range(nphase):
        m = 2 << j
        h = m // 2
        na = (NPIX + m - 1) // m          # active merge blocks
        nfull = NPIX // m                 # blocks fully below the boundary
        ain, aout = cur_in(), cur_out()
        i4 = ain.rearrange("p s (nb m) -> p s nb m", m=m)[:, :, 0:na]
        o4 = aout.rearrange("p s (nb m) -> p s nb m", m=m)[:, :, 0:na]

        def tri(blo, bhi, kmin, kmax):
            """Triangle compare-exchange for blocks [blo:bhi], min lanes
            [0:kmin], max lanes [0:kmax] (positions >= 196 are PADV at
            every stage and need no writes)."""
            ii = i4[:, :, blo:bhi]
            oo = o4[:, :, blo:bhi]
            if h > 1:
                hr = ii[:, :, :, m - 1:h - 1:-1]
                lr = ii[:, :, :, h - 1::-1]
            else:
                hr = ii[:, :, :, m - 1:m]
                lr = ii[:, :, :, 0:1]
            if kmin > 0:
                nc.vector.tensor_tensor(
                    oo[:, :, :, 0:kmin], ii[:, :, :, 0:kmin],
                    hr[:, :, :, 0:kmin], ALU.min)
            if kmax > 0:
                nc.vector.tensor_tensor(
                    oo[:, :, :, h:h + kmax], ii[:, :, :, h:h + kmax],
                    lr[:, :, :, 0:kmax], ALU.max)

        if nfull > 0:
            tri(0, nfull, h, h)
        if na > nfull:                    # boundary block, partial lanes
            base = nfull * m
            tri(nfull, na, min(h, NPIX - base),
                max(0, NPIX - base - h))
        cur_in, cur_out = cur_out, cur_in
        yield
        d = h // 2
        while d >= 1:
            # positions >= 196 hold PADV at EVERY stage (pads are global
            # maxima: each comparator's max lands at or above its inputs),
            # so sub-blocks based at >= 196 are droppable outright
            nc_act = (NPIX + 2 * d - 1) // (2 * d)
            ain, aout = cur_in(), cur_out()
            i4 = ain.rearrange("p s (nb t) -> p s nb t",
                               t=2 * d)[:, :, 0:nc_act]
            o4 = aout.rearrange("p s (nb t) -> p s nb t",
                                t=2 * d)[:, :, 0:nc_act]
            cmpex(o4[:, :, :, 0:d], o4[:, :, :, d:2 * d],
                  i4[:, :, :, 0:d], i4[:, :, :, d:2 * d],
                  i4[:, :, :, 0:d], i4[:, :, :, d:2 * d])
            cur_in, cur_out = cur_out, cur_in
            yield
            d //= 2
    assert cur_in == ping


def _build(dct_flat: tuple, nb: int = 3) -> bass.Bass:
    """Build the SPMD single-core program. dct_flat: 9 floats, row major."""
    D = np.asarray(dct_flat, np.float64).reshape(3, 3)
    nc = bacc.Bacc("TRN2", debug=False, enable_asserts=False)

    x_d = nc.dram_tensor("x", (B_CORE, C, H, W), F32, kind="ExternalInput")
    out_d = nc.dram_tensor("out", (B_CORE, HP, HP), F32, kind="ExternalOutput")
    xv = x_d.ap().rearrange("b c (hp i) (wp j) -> b c hp wp i j", i=PH, j=PH)
    ov = out_d.ap()

    segs = SEGS
    n_blocks = (segs + nb - 1) // nb
    d = [[float(D[r, c]) for c in range(3)] for r in range(3)]

    with TileContext(nc) as tc:
        with tc.tile_pool(name="persist", bufs=1) as pp:
            X = pp.tile([P, segs, PH, PH], F32)
            Xf = X.rearrange("p s i j -> p (s i j)")
            TMP = pp.tile([P, (segs // 8) * NPIX], F32)
            pdum = pp.tile([P, NWIN * NWIN], F32)
            lnd = pp.tile([P, NPIX], F32)
            psi_acc = pp.tile([P, segs], F32)
            e_acc = pp.tile([P, segs], F32)
            rich = pp.tile([P, segs], F32)
            rich3 = rich.rearrange("p (t c) -> p t c", c=C)
            tsum = pp.tile([P, segs // C], F32)
            osb = pp.tile([P, segs // C], F32)

            # ---- input DMAs: per (t, c, p1) a [32, 14, 14] strided load ----
            for t in range(T_BLKS):
                b = t // (T_BLKS // B_CORE)
                hp0 = (t % (T_BLKS // B_CORE)) * 4
                for c in range(C):
                    s = t * C + c
                    for p1 in range(4):
                        nc.sync.dma_start(
                            X[p1 * 32:(p1 + 1) * 32, s],
                            xv[b, c, hp0 + p1],
                        )
            # Per-seg absorber copies (each waits the 4 DMA queue sems of its
            # seg -- within the engine wait-queue depth); downstream reads of
            # X order behind these: DVE by program order, ACT via one
            # engine-counter semaphore.
            for t in range(T_BLKS):
                for c in range(C):
                    s = t * C + c
                    nc.vector.tensor_copy(X[:, s], X[:, s])

            ep_ctx = tc.tile_pool(name="ent", bufs=1)
            ep = ep_ctx.__enter__()
            SRT = ep.tile([P, segs, 1 + SW], BF16)
            b23p = pp.tile([P, 1], F32)
            b23n = pp.tile([P, 1], F32)
            nc.vector.memset(b23p, float(2 ** 23))
            nc.vector.memset(b23n, -float(2 ** 23))

            # quantize on ACT: codes = round(x*255) via the 2^23 RNE trick
            # (two Identity activations; Identity is in the same LUT set as
            # Ln/Exp/Square so no table reloads), into SRT cols 1..197
            # (bf16, exact for ints <= 256)
            TWO23 = float(2 ** 23)
            qch = (segs // 8) * NPIX
            TMP3 = TMP.rearrange("p (s k) -> p s k", k=NPIX)
            spq = segs // 8
            for q in range(8):
                # pass 1 (the RNE-rounding add) must be exact fp32 -> DVE;
                # pass 2 subtracts off 2^23 from integer-valued fp32 (exact
                # in any precision) -> ACT
                nc.gpsimd.tensor_scalar(
                    TMP, Xf[:, q * qch:(q + 1) * qch], 255.0, TWO23,
                    ALU.mult, ALU.add)
                nc.scalar.activation(
                    SRT[:, q * spq:(q + 1) * spq, 1:1 + NPIX], TMP3,
                    ACTF.Identity, bias=b23n)
            # sentinel col 0 = -1; pad cols 197..256 = PADV
            nc.vector.memset(SRT[:, :, 0:1], -1.0)
            nc.vector.memset(SRT[:, :, 1 + NPIX:1 + SW], PADV)

            # ============ conv/psi blocks with interleaved sort ============
            wp_ctx = tc.tile_pool(name="work", bufs=2)
            wp = wp_ctx.__enter__()
            Bp = wp.tile([P, segs, SW], BF16, tag="BP", name="BP", bufs=1)
            # pads in the pong buffer too: the trailing all-pad merge
            # blocks are skipped by every sort stage, so both buffers
            # must carry PADV there from the start
            nc.gpsimd.memset(Bp[:, :, NPIX:SW], PADV)
            sorter = _sort_stages(nc, SRT, Bp)
            sort_left = 36

            def emit_sort(k):
                nonlocal sort_left
                for _ in range(min(k, sort_left)):
                    next(sorter)
                    sort_left -= 1

            GROUPS = (
                [[(r, 0), (r, 1), (r, 2)] for r in range(3)]       # rows
                + [[(0, c), (1, c), (2, c)] for c in range(3)]     # cols
                + [[(0, 0), (1, 1), (2, 2)],                       # diag
                   [(0, 2), (1, 1), (2, 0)]]                       # anti
            )

            def emit_psi(st):
                """psi for a previous block -- emitted one block late so the
                next block's convs aren't queued behind the psi stalls.
                fp32 temps alias the M8 tiles (last read by the VARMs) plus
                a dedicated 5-slot fp32 scratch; sig (bf16) is read-only so
                the sum2 - A^2/4 cancellation stays consistent in fp32."""
                s0, sn, M8, SS, PT = st
                u1, u2 = M8[0], M8[1]
                tb1, tb2 = M8[2], M8[3]
                ab, s2b = M8[4], M8[5]
                aqb, stb = M8[6], M8[7]
                s6q, s7q = PT[:, :, 0], PT[:, :, 1]
                # stb -> ssqb -> rb -> psib all reuse one slot in place
                stb = ssqb = rb = psib = PT[:, :, 2]
                sig = SS

                nc.gpsimd.tensor_add(u1, sig[0], sig[1])
                nc.gpsimd.tensor_add(u1, u1, sig[2])
                nc.gpsimd.tensor_add(u2, sig[3], sig[4])
                nc.gpsimd.tensor_add(u2, u2, sig[5])
                # A = U1/3 + U2/3 + sig6 + sig7
                nc.vector.scalar_tensor_tensor(
                    tb1, u1, 1.0 / 3, sig[6], ALU.mult, ALU.add)
                nc.vector.scalar_tensor_tensor(
                    tb2, u2, 1.0 / 3, sig[7], ALU.mult, ALU.add)
                nc.gpsimd.tensor_add(ab, tb1, tb2)
                # sum of squared directional stds
                nc.scalar.activation(u1, u1, ACTF.Square, scale=1.0 / 3)
                nc.scalar.activation(u2, u2, ACTF.Square, scale=1.0 / 3)
                nc.scalar.activation(s6q, sig[6], ACTF.Square)
                nc.scalar.activation(s7q, sig[7], ACTF.Square)
                nc.gpsimd.tensor_add(tb1, u1, u2)
                nc.gpsimd.tensor_add(tb2, s6q, s7q)
                nc.gpsimd.tensor_add(s2b, tb1, tb2)
                # psi = (sum2 - A^2/4) / (3 * (A/4 + 1e-8)^2)
                nc.scalar.activation(aqb, ab, ACTF.Square, scale=0.5)
                nc.gpsimd.tensor_sub(s2b, s2b, aqb)
                nc.vector.tensor_scalar(
                    stb, ab, 0.25, 1e-8, ALU.mult, ALU.add)
                nc.scalar.activation(ssqb, stb, ACTF.Square)
                nc.vector.reciprocal(rb, ssqb)
                nc.vector.scalar_tensor_tensor(
                    psib, s2b, 1.0 / 3, rb, ALU.mult, ALU.mult)
                # psi_m accumulate per seg on ACT (Identity + accum_out)
                for i in range(sn):
                    s = s0 + i
                    nc.scalar.activation(
                        pdum, psib[:, i], ACTF.Identity,
                        accum_out=psi_acc[:, s:s + 1])

            # run-length scan chunks (6 segs each, 8 chunks) emitted as DVE
            # filler into the last blocks once the sort has finished
            cs = segs // 8

            def emit_scan_chunk(h):
                s0c = h * cs
                src0 = SRT[:, s0c:s0c + cs, 1:1 + SDOM]
                src1 = SRT[:, s0c:s0c + cs, 0:SDOM]
                SC = wp.tile([P, cs * SDOM], F32, tag="SCS", name="SCS",
                             bufs=2)
                MC = wp.tile([P, cs * SDOM], F32, tag="SCM", name="SCM",
                             bufs=1)
                NX = wp.tile([P, cs * SDOM], F32, tag="SCN", name="SCN",
                             bufs=1)
                nc.vector.ath_bidx_neg(MC, src0, src1)
                nc.vector.ath_scan_max(SC, MC)
                nc.vector.ath_bidx_pos(MC, src0, src1)
                n = cs * SDOM
                nc.vector.ath_scan_min(NX[:, n - 1::-1], MC[:, n - 1::-1],
                                       BIG)
                Svc = SC.rearrange("p (s k) -> p s k", k=SDOM)
                NXv = NX.rearrange("p (s k) -> p s k", k=SDOM)
                # c[f] = NXT[f+1] - S[f], in place (codes rows only)
                nc.gpsimd.tensor_tensor(
                    Svc[:, :, 0:NPIX], NXv[:, :, 1:1 + NPIX],
                    Svc[:, :, 0:NPIX], ALU.subtract)
                for i in range(cs):
                    nc.scalar.activation(
                        lnd, Svc[:, i, 0:NPIX], ACTF.Ln,
                        accum_out=e_acc[:, s0c + i:s0c + i + 1])

            prev = None
            for blk in range(n_blocks):
                s0 = blk * nb
                sn = min(nb, segs - s0)
                assert sn == nb, "segs must divide by nb"
                V = [wp.tile([P, nb, NWIN, PH], BF16, tag=f"V{r}",
                             name=f"V{r}", bufs=1) for r in range(3)]
                Y = [[wp.tile([P, nb, NWIN * NWIN], BF16, tag=f"Y{r}{c}",
                              name=f"Y{r}{c}")
                      for c in range(3)] for r in range(3)]
                xbf = X[:, s0:s0 + sn].rearrange("p n i j -> p n (i j)")

                # convs: V and Y in bf16 (horizontal third taps then run at
                # the DVE 2x rate; SS and M derive from the same rounded Y
                # so the rounding cancels in sigma^2)
                with nc.allow_low_precision(
                        reason="bf16 V/Y conv planes; SS and M derive from "
                               "the same rounded Y so rounding cancels in "
                               "sigma^2"):
                    # vertical convs V_r(i,j) = sum_k D[r,k] x(i+k, j) on
                    # rank-3 coalesced views of the flat (i j) layout
                    for r in range(3):
                        vf = V[r].rearrange("p n i j -> p n (i j)")
                        nc.vector.ath_conv2(
                            vf, xbf[:, :, 0:NWIN * PH],
                            xbf[:, :, PH:PH * 13], d[r][0], d[r][1])
                        nc.vector.scalar_tensor_tensor(
                            vf, xbf[:, :, 2 * PH:PH * 14], d[r][2], vf,
                            ALU.mult, ALU.add)
                    # horizontal convs Y_rc(i,j) = sum_l D[c,l] V_r(i, j+l)
                    # on (n i)-coalesced rank-3 views
                    for r in range(3):
                        v3 = V[r].rearrange("p n i j -> p (n i) j")
                        for c in range(3):
                            y3 = Y[r][c].rearrange(
                                "p n (i j) -> p (n i) j", j=NWIN)
                            nc.vector.ath_conv2(
                                y3, v3[:, :, 0:NWIN], v3[:, :, 1:1 + NWIN],
                                d[c][0], d[c][1])
                            nc.vector.scalar_tensor_tensor(
                                y3, v3[:, :, 2:2 + NWIN], d[c][2], y3,
                                ALU.mult, ALU.add)

                M8T = wp.tile([P, nb, 8, NWIN * NWIN], F32, tag="M8T",
                              name="M8T")
                SST = wp.tile([P, nb, 8, NWIN * NWIN], BF16, tag="SST",
                              name="SST", bufs=1)
                M8 = [M8T[:, :, g] for g in range(8)]
                SS = [SST[:, :, g] for g in range(8)]
                # squares of Y (ACT) -- queued ahead of Msq so the SS sums
                # unblock early; bf16 so the SS sums run at the DVE 2x rate
                # (sigma^2 = SS/3 - Msq stays safe: SS and M derive from the
                # same rounded Y, so rounding mostly cancels in the variance)
                sq = wp.tile([P, nb, 9, NWIN * NWIN], BF16, tag="SQ",
                             name="sq", bufs=1)
                for r in range(3):
                    for c in range(3):
                        nc.scalar.activation(sq[:, :, r * 3 + c], Y[r][c],
                                             ACTF.Square)
                # psi of the previous block fills the conv->Msq gap
                if prev is not None:
                    emit_psi(prev)
                # group sums of Y: first pair-add on DVE in bf16 (2x, cheap),
                # final add on Pool -- halves the Pool chain ahead of
                # Msq -> VARM -> Sqrt; Msq = (M/3)^2 (ACT)
                mt = wp.tile([P, nb, 8, NWIN * NWIN], BF16, tag="MT",
                             name="mt", bufs=1)
                with nc.allow_low_precision(
                        reason="bf16 partial group sums; one extra rounding "
                               "on M, averaged out over 144 windows"):
                    for g, mem in enumerate(GROUPS):
                        nc.vector.tensor_add(
                            mt[:, :, g], Y[mem[0][0]][mem[0][1]],
                            Y[mem[1][0]][mem[1][1]])
                for g, mem in enumerate(GROUPS):
                    mb = M8[g]
                    nc.gpsimd.tensor_add(mb, mt[:, :, g],
                                         Y[mem[2][0]][mem[2][1]])
                    nc.scalar.activation(mb, mb, ACTF.Square, scale=1.0 / 3)
                emit_sort(1)
                with nc.allow_low_precision(
                        reason="SS sums in bf16: sigma^2 = SS/3 - Msq uses "
                               "the same rounded Y on both sides, rounding "
                               "cancels in the variance; errors average out "
                               "over 144 windows"):
                    for g, mem in enumerate(GROUPS):
                        sb = SS[g]
                        nc.vector.tensor_add(
                            sb, sq[:, :, mem[0][0] * 3 + mem[0][1]],
                            sq[:, :, mem[1][0] * 3 + mem[1][1]])
                        nc.vector.tensor_add(
                            sb, sb, sq[:, :, mem[2][0] * 3 + mem[2][1]])
                    # sigma^2 = max(SS/3 - Msq, eps) and sigma = sqrt, each
                    # as ONE instruction over the contiguous 8-group tiles
                    # (same LUT set as Square -- one table switch, at the
                    # final Ln)
                    SSf = SST.rearrange("p n g k -> p n (g k)")
                    M8f = M8T.rearrange("p n g k -> p n (g k)")
                    nc.vector.ath_varm(SSf, SSf, M8f, 1.0 / 3, 1e-38)
                    nc.scalar.activation(SSf, SSf, ACTF.Sqrt)
                emit_sort(1)
                if blk >= 12:
                    emit_sort(1)
                PT = wp.tile([P, nb, 3, NWIN * NWIN], F32, tag="PT",
                             name="PT")
                prev = (s0, sn, M8, SS, PT)

            emit_psi(prev)
            emit_sort(36)
            for h in range(8):
                emit_scan_chunk(h)
            wp_ctx.__exit__(None, None, None)
            ep_ctx.__exit__(None, None, None)

            # ---- richness = psi_m * entropy, mean over channels ----
            nc.vector.tensor_scalar(
                e_acc, e_acc, -1.0 / (NPIX * LN2), float(math.log2(NPIX)),
                ALU.mult, ALU.add)
            nc.vector.scalar_tensor_tensor(
                rich, psi_acc, 1.0 / (NWIN * NWIN), e_acc,
                ALU.mult, ALU.mult)
            nc.vector.tensor_add(tsum, rich3[:, :, 0], rich3[:, :, 1])
            nc.vector.tensor_add(tsum, tsum, rich3[:, :, 2])
            nc.vector.tensor_scalar(osb, tsum, 1.0 / C, None, ALU.mult)

            # ---- output DMAs ----
            for t in range(T_BLKS):
                b = t // (T_BLKS // B_CORE)
                hp0 = (t % (T_BLKS // B_CORE)) * 4
                nc.sync.dma_start(ov[b, hp0:hp0 + 4], osb[:, t:t + 1])

    nc.compile()
    return nc


@functools.lru_cache(maxsize=4)
def _build_cached(dct_flat: tuple) -> bass.Bass:
    return _build(dct_flat)


def kernel(x, dct_matrix):
    x = np.ascontiguousarray(np.asarray(x, dtype=np.float32))
    D = np.asarray(dct_matrix, dtype=np.float32)
    assert x.shape == (B_FULL, C, H, W), x.shape
    nc = _build_cached(tuple(float(v) for v in D.flatten()))
    in_maps = [
        {"x": np.ascontiguousarray(x[i * B_CORE:(i + 1) * B_CORE])}
        for i in range(N_CORES)
    ]
    res = bass_utils.run_bass_kernel_spmd(
        nc, in_maps, core_ids=list(range(N_CORES)))
    out = np.concatenate([r["out"] for r in res.results], axis=0)
    return out.astype(np.float32)
